# revision 19
# baseline (speedup 1.0000x reference)
"""Trainium2 Bass kernel for nn_Attention_41085657153633.

B=8, N=1024, C=384, H=6, D=64 attention with per-head q/k normalization
(mean/std over head_dim, ddof=1), softmax(QK^T/sqrt(D)) @ V, output proj.

Sharding: data-parallel over B — each of the 8 NeuronCores handles one
batch element end-to-end (no collectives).

Per-core dataflow (all matmul operands bf16; fp32 PSUM accumulation):
  - host supplies x[b]^T, qkv_w^T, proj_w^T pre-cast to bf16 (halves HBM
    traffic, enables fast weight load on the PE).
  - QKV^T computed head-major for Q,K ([d on partitions, tokens on free]
    — the layout QK^T wants) and token-major for V (the AV stationary).
    V is stored as [v_h | 64 ones cols] per head so AV rows 64-127
    accumulate the softmax denominator broadcast across 64 partitions
    for free in the matmul M dimension.
  - q/k normalization over the partition (d) axis via block-diagonal
    ones matmuls: mean_bcast = blockdiag(1/64) @ q and ssq_bcast =
    blockdiag(1/64) @ (q-mean)^2 come out broadcast across each head's
    64 partitions by construction; rstd = exp(-0.5*ln(ssq) + b) with the
    ddof=1 correction folded into the ACT bias (ln/exp share one ACT
    table set with the softmax exp — zero table switches after load).
  - scores S^T[s,t] per head via K^T x Q^T (contraction over d=64);
    head pairs occupy PE row groups T0/T8, alternating so each weight
    load overlaps the other group's matmul.
  - softmax denominators inverted as exp(-ln d) on ACT (ln/exp/softmax
    exp share one table set — no table switches in the whole kernel).
  - scheduling: the norm pipeline is interleaved into the QKV/V matmul
    stream, and each pair's AV accumulation is interleaved per token
    chunk with the NEXT pair's QK^T, so the PE never idles long enough
    for the HAM clock gate to re-throttle to 4/8.
  - softmax max-subtraction skipped: |S/8| <~ 7, exp stays in range.
  - output proj from attn_out^T; bf16 result DMA'd out, host casts.
"""

import sys

sys.path.insert(0, "/opt/trn_rl_repo")

import json

import numpy as np

B, N, C = 8, 1024, 384
H, D = 6, 64
NCORES = 8

_prog = None


def _install_multiwait_fixup():
    """This container's walrus build rejects >1 sync wait per instruction
    ("Too many sync wait commands"). Rewrite the BIR JSON before compile:
    hoist extra waits onto single-wait EventSemaphore instructions
    inserted just before the owner on the same engine (engines dispatch
    in program order, so the gating is preserved)."""
    from concourse import bass2jax, bass_utils

    if getattr(bass_utils, "_multiwait_fixup", False):
        return
    bass_utils._multiwait_fixup = True

    orig = bass_utils.compile_bir_kernel

    def _split(bir_json: bytes) -> bytes:
        j = json.loads(bir_json)
        for fn in j.get("functions", []):
            for bb in fn.get("blocks", []):
                out = []
                for inst in bb.get("instructions", []):
                    si = inst.get("sync_info")
                    waits = si.get("on_wait", []) if si else []
                    if len(waits) > 1:
                        for k, w in enumerate(waits[:-1]):
                            out.append({
                                "debug": inst.get("debug", 0),
                                "engine": inst["engine"],
                                "ins": [],
                                "outs": [],
                                "name": f"{inst['name']}-sw{k}",
                                "opcode": "EventSemaphore",
                                "sync_info": {"on_update": [], "on_wait": [w]},
                            })
                        si["on_wait"] = [waits[-1]]
                    out.append(inst)
                bb["instructions"] = out
        return json.dumps(j).encode()

    def patched(bir_json, tmpdir, neff_name="file.neff"):
        return orig(_split(bir_json), tmpdir, neff_name)

    bass_utils.compile_bir_kernel = patched
    bass2jax.compile_bir_kernel = patched


def _build():
    import concourse.bass as bass
    import concourse.tile as tile
    from concourse import mybir

    _install_multiwait_fixup()

    F32 = mybir.dt.float32
    BF16 = mybir.dt.bfloat16
    EXP = mybir.ActivationFunctionType.Exp
    LN = mybir.ActivationFunctionType.Ln
    COPY = mybir.ActivationFunctionType.Copy
    RSQRT = mybir.ActivationFunctionType.Rsqrt

    nc = bass.Bass("TRN2")
    xT = nc.dram_tensor("xT", [C, N], BF16, kind="ExternalInput")
    qkv_wT = nc.dram_tensor("qkv_wT", [C, 3 * C], BF16, kind="ExternalInput")
    proj_wT = nc.dram_tensor("proj_wT", [C, C], BF16, kind="ExternalInput")
    pb = nc.dram_tensor("pb", [128, 3], F32, kind="ExternalInput")
    bd_in = nc.dram_tensor("bd_in", [128, 128], BF16, kind="ExternalInput")
    outT = nc.dram_tensor("outT", [C, N], BF16, kind="ExternalOutput")

    KC = C // 128   # 3 contraction chunks of the model dim
    TC = N // 128   # 8 token chunks
    scale = float(D) ** -0.5
    order = [0, 3, 1, 4, 2, 5]   # q0,k0,q1,k1,q2,k2 chunk emission order

    with tile.TileContext(nc) as tc:
      with nc.allow_low_precision(reason="bf16 matmul intermediates"):
        with tc.tile_pool(name="consts", bufs=1) as consts, \
             tc.tile_pool(name="ins", bufs=1) as ins, \
             tc.tile_pool(name="persist", bufs=1) as persist, \
             tc.tile_pool(name="work", bufs=2) as work, \
             tc.tile_pool(name="es", bufs=16) as esp, \
             tc.tile_pool(name="avn", bufs=2) as avn, \
             tc.tile_pool(name="po", bufs=2) as pop, \
             tc.tile_pool(name="ps", bufs=2, space="PSUM") as ps:

            # ---- constants + input DMA ----
            bd = consts.tile([128, 128], BF16)
            pbt = consts.tile([128, 3], F32)
            ddof_b = consts.tile([128, 1], F32)
            nc.vector.memset(ddof_b[:], -0.5 * float(np.log(64.0 / 63.0)))

            xt = ins.tile([128, KC, N], BF16)
            wq = ins.tile([128, KC, 3 * C], BF16)
            wp = ins.tile([128, KC, C], BF16)

            # HAM warmup gated only on the tiny ddof memset — no DMA
            # dependency, so the PE clock gate starts opening immediately.
            warm_ps = ps.tile([64, 512], F32, tag="s")
            warm_rhs = bass.AP(
                tensor=ddof_b.tensor, offset=ddof_b.offset,
                ap=[list(ddof_b.ap[0]), [0, 512]])  # [128, 512] step-0
            for _ in range(12):
                nc.tensor.matmul(warm_ps[0:1, :], ddof_b[:, 0:1], warm_rhs,
                                 start=True, stop=True)

            xr = xT.rearrange("(k p) n -> p k n", p=128)
            wr = qkv_wT.rearrange("(k p) m -> p k m", p=128)
            nc.scalar.dma_start(out=xt[:], in_=xr[:])
            for k in range(KC):
                nc.sync.dma_start(out=wq[:, k, 0:128], in_=wr[:, k, 0:128])
                nc.sync.dma_start(out=wq[:, k, 384:512], in_=wr[:, k, 384:512])
            nc.sync.dma_start(out=bd[:], in_=bd_in[:, :])
            nc.sync.dma_start(out=pbt[:], in_=pb[:, :])
            for k in range(KC):
                nc.sync.dma_start(out=wq[:, k, 128:384], in_=wr[:, k, 128:384])
                nc.sync.dma_start(out=wq[:, k, 512:1152], in_=wr[:, k, 512:1152])
            nc.sync.dma_start(
                out=wp[:], in_=proj_wT.rearrange("(k p) m -> p k m", p=128))

            vo = persist.tile([128, TC, H, 128], BF16)
            nc.vector.memset(vo[:, :, :, D:128], 1.0)

            qn = persist.tile([128, 2 * H, N], BF16)   # normalized q|k
            aoT = persist.tile([128, KC, N], BF16)     # attn out (proj moving)

            # ---- phase B emitters: QKV chunks + pipelined normalization ----
            qk_ps = {}      # chunk j -> psum tile with raw q|k rows
            qkr = {}        # chunk j -> bf16 copy in SBUF
            mean_ps = {}
            qc = {}
            qc2 = {}
            ssq_ps = {}

            def emit_qkv(j):
                p = ps.tile([128, N], F32, tag="s", name=f"qk_ps{j}")
                qk_ps[j] = p
                for k in range(KC):
                    for h5 in range(2):
                        nc.tensor.matmul(
                            p[:, h5 * 512:(h5 + 1) * 512],
                            wq[:, k, j * 128:(j + 1) * 128],
                            xt[:, k, h5 * 512:(h5 + 1) * 512],
                            start=(k == 0), stop=(k == KC - 1))
                # PSUM->SBUF bf16 copy on DVE (ACT is the busiest engine)
                q = work.tile([128, N], BF16, tag="qkr", name=f"qkr{j}")
                nc.vector.tensor_copy(q[:], p[:])
                qkr[j] = q

            def emit_v(t):
                v_ps = ps.tile([128, C], F32, tag="s", name=f"v_ps{t}")
                for k in range(KC):
                    nc.tensor.matmul(
                        v_ps[:],
                        xt[:, k, t * 128:(t + 1) * 128],
                        wq[:, k, 2 * C:3 * C],
                        start=(k == 0), stop=(k == KC - 1))
                nc.vector.tensor_copy(
                    vo[:, t, :, 0:D],
                    v_ps[:].rearrange("p (h d) -> p h d", h=H))

            def emit_mean(j):
                """mean broadcast + centered q, squared (DVE chain)."""
                m = ps.tile([128, N], F32, tag="av", name=f"mean_ps{j}")
                mean_ps[j] = m
                for h5 in range(2):
                    nc.tensor.matmul(m[:, h5 * 512:(h5 + 1) * 512],
                                     bd[:], qkr[j][:, h5 * 512:(h5 + 1) * 512],
                                     start=True, stop=True)
                c = work.tile([128, N], BF16, tag="qc", name=f"qc{j}")
                nc.vector.tensor_sub(c[:], qkr[j][:], m[:])
                qc[j] = c
                c2 = work.tile([128, N], BF16, tag="qc2", name=f"qc2{j}")
                nc.vector.tensor_mul(c2[:], c[:], c[:])
                qc2[j] = c2

            def emit_ssq(j):
                """ssq broadcast -> rstd (ACT ln/exp; Rsqrt is blocked in
                this bass build) -> qn (DVE mul)."""
                s = ps.tile([128, N], F32, tag="av", name=f"ssq_ps{j}")
                ssq_ps[j] = s
                for h5 in range(2):
                    nc.tensor.matmul(s[:, h5 * 512:(h5 + 1) * 512],
                                     bd[:], qc2[j][:, h5 * 512:(h5 + 1) * 512],
                                     start=True, stop=True)
                lnv = work.tile([128, N], F32, tag="lnv", name=f"lnv{j}")
                nc.scalar.activation(lnv[:], s[:], LN)
                rstd = work.tile([128, N], BF16, tag="rstd", name=f"rstd{j}")
                nc.scalar.activation(rstd[:], lnv[:], EXP, scale=-0.5,
                                     bias=ddof_b[:])
                nc.vector.tensor_mul(qn[:, j, :], qc[j][:], rstd[:])

            # pipeline: QKV(jj) | V(jj) | mean(jj-1) | ssq(jj-2)
            for jj in range(6):
                emit_qkv(order[jj])
                emit_v(jj)
                if jj >= 1:
                    emit_mean(order[jj - 1])
                if jj >= 2:
                    emit_ssq(order[jj - 2])

            # ---- pairs: QK^T -> exp -> AV, cross-pair interleaved ----
            es_tiles = {}   # (pair, t, p) -> bf16 exp(scores) tile

            def emit_qk_t(j, t):
                """Scores S^T for heads 2j,2j+1, token chunk t (64-row PE
                mode, groups T0/T8 alternating) + the exp on ACT."""
                s_tiles = {}
                for p in range(2):
                    s_tiles[p] = ps.tile([128, N], F32, tag="s",
                                         name=f"s{j}_{t}_{p}")
                for h5 in range(2):
                    for p in range(2):
                        lo = p * 64
                        nc.tensor.matmul(
                            s_tiles[p][:, h5 * 512:(h5 + 1) * 512],
                            qn[lo:lo + 64, 3 + j, t * 128:(t + 1) * 128],
                            qn[lo:lo + 64, j, h5 * 512:(h5 + 1) * 512],
                            start=True, stop=True)
                for p in range(2):
                    es = esp.tile([128, N], BF16, tag="es",
                                  name=f"es{j}_{t}_{p}")
                    nc.scalar.activation(es[:], s_tiles[p][:], EXP, scale=scale)
                    es_tiles[(j, t, p)] = es

            av_ps = {}

            def emit_av_t(j, t):
                """AV accumulation step t for pair j (128-row mode)."""
                for p in range(2):
                    if t == 0:
                        av_ps[(j, p)] = ps.tile([128, N], F32, tag="av",
                                                name=f"av{j}_{p}")
                    for h5 in range(2):
                        nc.tensor.matmul(
                            av_ps[(j, p)][:, h5 * 512:(h5 + 1) * 512],
                            vo[:, t, 2 * j + p, :],
                            es_tiles[(j, t, p)][:, h5 * 512:(h5 + 1) * 512],
                            start=(t == 0), stop=(t == TC - 1))

            def emit_norm_out(j):
                """1/denominator = exp(-ln d) on ACT, attn-out scale on DVE.
                Rows 64-127 of av_ps hold the softmax denominator already
                broadcast across 64 partitions (ones columns of vo)."""
                for p in range(2):
                    a = av_ps[(j, p)]
                    lnd = avn.tile([64, N], F32, tag="lnd", name=f"lnd{j}{p}")
                    nc.scalar.activation(lnd[:], a[D:128, :], LN)
                    rec = avn.tile([64, N], F32, tag="rec", name=f"rec{j}{p}")
                    nc.scalar.activation(rec[:], lnd[:], EXP, scale=-1.0)
                    lo = p * 64
                    nc.vector.tensor_mul(aoT[lo:lo + 64, j, :],
                                         a[0:D, :], rec[:])

            # phase B tail: all remaining Rsqrt work BEFORE the first exp
            # (one table-set switch total), then pair-0 QK^T fills the PE
            # while the last norm chains drain and the exp tables load.
            emit_v(6)
            emit_mean(order[5])
            emit_qk_t(0, 0)
            emit_v(7)
            emit_ssq(order[4])
            emit_qk_t(0, 1)
            emit_ssq(order[5])
            emit_qk_t(0, 2)
            emit_qk_t(0, 3)

            # global QK/AV software pipeline across all pairs, AV lagging 4
            # token-chunks behind QK: the PE always has a ready matmul while
            # ACT streams the exps, and pair boundaries have no PE gap.
            steps = [(j, t) for j in range(3) for t in range(TC)]
            LAG = 5
            proj_ps = {}
            for s in range(4, len(steps) + LAG):
                if s < len(steps):
                    emit_qk_t(*steps[s])
                if s < LAG:
                    continue
                if s == len(steps) + LAG - 2:
                    # pre-accumulate proj contractions k=0,1 for the first
                    # two output chunks (pairs 0,1 attn-out already final);
                    # co=2 stays fully in the tail so the 2-slot "s" tag
                    # rotation can't WAR-deadlock the in-order PE queue.
                    for co in range(2):
                        p_ps = ps.tile([128, N], F32, tag="s",
                                       name=f"p_ps{co}")
                        proj_ps[co] = p_ps
                        for h5 in range(2):
                            for k in range(2):
                                nc.tensor.matmul(
                                    p_ps[:, h5 * 512:(h5 + 1) * 512],
                                    wp[:, k, co * 128:(co + 1) * 128],
                                    aoT[:, k, h5 * 512:(h5 + 1) * 512],
                                    start=(k == 0), stop=False)
                j, t = steps[s - LAG]
                emit_av_t(j, t)
                if t == TC - 1:
                    emit_norm_out(j)

            # ---- output projection tail ----
            for co in range(2):
                p_ps = proj_ps[co]
                for h5 in range(2):
                    nc.tensor.matmul(
                        p_ps[:, h5 * 512:(h5 + 1) * 512],
                        wp[:, 2, co * 128:(co + 1) * 128],
                        aoT[:, 2, h5 * 512:(h5 + 1) * 512],
                        start=False, stop=True)
                po = pop.tile([128, N], BF16, tag="po", name=f"po{co}")
                nc.vector.tensor_scalar_add(po[:], p_ps[:], pbt[:, co:co + 1])
                nc.sync.dma_start(out=outT[co * 128:(co + 1) * 128, :], in_=po[:])
            p_ps2 = ps.tile([128, N], F32, tag="s", name="p_ps2")
            for h5 in range(2):
                for k in range(KC):
                    nc.tensor.matmul(
                        p_ps2[:, h5 * 512:(h5 + 1) * 512],
                        wp[:, k, 2 * 128:3 * 128],
                        aoT[:, k, h5 * 512:(h5 + 1) * 512],
                        start=(k == 0), stop=(k == KC - 1))
            po2 = pop.tile([128, N], BF16, tag="po", name="po2")
            nc.vector.tensor_scalar_add(po2[:], p_ps2[:], pbt[:, 2:3])
            nc.sync.dma_start(out=outT[2 * 128:3 * 128, :], in_=po2[:])

    return nc


def _get_prog():
    global _prog
    if _prog is None:
        _prog = _build()
    return _prog


def _make_in_maps(x, qkv_w, proj_w, proj_b):
    from ml_dtypes import bfloat16

    qkv_wT = np.ascontiguousarray(np.asarray(qkv_w, np.float32).T).astype(bfloat16)
    proj_wT = np.ascontiguousarray(np.asarray(proj_w, np.float32).T).astype(bfloat16)
    pb = np.ascontiguousarray(
        np.asarray(proj_b, np.float32).reshape(3, 128).T)
    bd_in = np.zeros((128, 128), np.float32)
    for b0 in (0, 64):
        bd_in[b0:b0 + 64, b0:b0 + 64] = 1.0 / D   # ddof fix in rstd exp bias
    bd_in = bd_in.astype(bfloat16)

    shared = {
        "qkv_wT": qkv_wT, "proj_wT": proj_wT, "pb": pb, "bd_in": bd_in,
    }
    x = np.asarray(x, np.float32)
    return [
        {"xT": np.ascontiguousarray(x[b].T).astype(bfloat16), **shared}
        for b in range(B)
    ]


def run(x, qkv_w, proj_w, proj_b, trace=False):
    from concourse.bass_utils import run_bass_kernel_spmd

    nc = _get_prog()
    in_maps = _make_in_maps(x, qkv_w, proj_w, proj_b)
    res = run_bass_kernel_spmd(
        nc, in_maps, core_ids=list(range(NCORES)), trace=trace)
    out = np.stack(
        [res.results[b]["outT"].astype(np.float32).T for b in range(B)])
    return np.ascontiguousarray(out.astype(np.float32)), res


def kernel(x, qkv_w, proj_w, proj_b):
    out, _ = run(x, qkv_w, proj_w, proj_b)
    return out


# revision 21
# speedup vs baseline: 1.1550x; 1.1550x over previous
"""Trainium2 Bass kernel for nn_Attention_41085657153633.

B=8, N=1024, C=384, H=6, D=64 attention with per-head q/k normalization
(mean/std over head_dim, ddof=1), softmax(QK^T/sqrt(D)) @ V, output proj.

Sharding: data-parallel over B — each of the 8 NeuronCores handles one
batch element end-to-end (no collectives).

Per-core dataflow (all matmul operands bf16; fp32 PSUM accumulation):
  - host supplies x[b]^T, qkv_w^T, proj_w^T pre-cast to bf16 (halves HBM
    traffic, enables fast weight load on the PE).
  - QKV^T computed head-major for Q,K ([d on partitions, tokens on free]
    — the layout QK^T wants) and token-major for V (the AV stationary).
    V is stored as [v_h | 64 ones cols] per head so AV rows 64-127
    accumulate the softmax denominator broadcast across 64 partitions
    for free in the matmul M dimension.
  - q/k normalization over the partition (d) axis via block-diagonal
    ones matmuls: mean_bcast = blockdiag(1/64) @ q and ssq_bcast =
    blockdiag(1/64) @ (q-mean)^2 come out broadcast across each head's
    64 partitions by construction; rstd = exp(-0.5*ln(ssq) + b) with the
    ddof=1 correction folded into the ACT bias (ln/exp share one ACT
    table set with the softmax exp — zero table switches after load).
  - scores S^T[s,t] per head via K^T x Q^T (contraction over d=64);
    head pairs occupy PE row groups T0/T8, alternating so each weight
    load overlaps the other group's matmul.
  - softmax denominators inverted as exp(-ln d) on ACT (ln/exp/softmax
    exp share one table set — no table switches in the whole kernel).
  - scheduling: the norm pipeline is interleaved into the QKV/V matmul
    stream, and each pair's AV accumulation is interleaved per token
    chunk with the NEXT pair's QK^T, so the PE never idles long enough
    for the HAM clock gate to re-throttle to 4/8.
  - softmax max-subtraction skipped: |S/8| <~ 7, exp stays in range.
  - output proj from attn_out^T; bf16 result DMA'd out, host casts.
"""

import sys

sys.path.insert(0, "/opt/trn_rl_repo")

import json

import numpy as np

B, N, C = 8, 1024, 384
H, D = 6, 64
NCORES = 8

_prog = None


def _install_multiwait_fixup():
    """This container's walrus build rejects >1 sync wait per instruction
    ("Too many sync wait commands"). Rewrite the BIR JSON before compile:
    hoist extra waits onto single-wait EventSemaphore instructions
    inserted just before the owner on the same engine (engines dispatch
    in program order, so the gating is preserved)."""
    from concourse import bass2jax, bass_utils

    if getattr(bass_utils, "_multiwait_fixup", False):
        return
    bass_utils._multiwait_fixup = True

    orig = bass_utils.compile_bir_kernel

    def _split(bir_json: bytes) -> bytes:
        j = json.loads(bir_json)
        for fn in j.get("functions", []):
            for bb in fn.get("blocks", []):
                out = []
                for inst in bb.get("instructions", []):
                    si = inst.get("sync_info")
                    waits = si.get("on_wait", []) if si else []
                    if len(waits) > 1:
                        for k, w in enumerate(waits[:-1]):
                            out.append({
                                "debug": inst.get("debug", 0),
                                "engine": inst["engine"],
                                "ins": [],
                                "outs": [],
                                "name": f"{inst['name']}-sw{k}",
                                "opcode": "EventSemaphore",
                                "sync_info": {"on_update": [], "on_wait": [w]},
                            })
                        si["on_wait"] = [waits[-1]]
                    out.append(inst)
                bb["instructions"] = out
        return json.dumps(j).encode()

    def patched(bir_json, tmpdir, neff_name="file.neff"):
        return orig(_split(bir_json), tmpdir, neff_name)

    bass_utils.compile_bir_kernel = patched
    bass2jax.compile_bir_kernel = patched


def _build():
    import concourse.bass as bass
    import concourse.tile as tile
    from concourse import mybir

    _install_multiwait_fixup()

    F32 = mybir.dt.float32
    BF16 = mybir.dt.bfloat16
    EXP = mybir.ActivationFunctionType.Exp
    LN = mybir.ActivationFunctionType.Ln
    COPY = mybir.ActivationFunctionType.Copy
    RSQRT = mybir.ActivationFunctionType.Rsqrt

    nc = bass.Bass("TRN2")
    xT = nc.dram_tensor("xT", [C, N], BF16, kind="ExternalInput")
    qkv_wT = nc.dram_tensor("qkv_wT", [C, 3 * C], BF16, kind="ExternalInput")
    proj_wT = nc.dram_tensor("proj_wT", [C, C], BF16, kind="ExternalInput")
    pb = nc.dram_tensor("pb", [128, 3], F32, kind="ExternalInput")
    bd_in = nc.dram_tensor("bd_in", [128, 128], BF16, kind="ExternalInput")
    outT = nc.dram_tensor("outT", [C, N], BF16, kind="ExternalOutput")

    KC = C // 128   # 3 contraction chunks of the model dim
    TC = N // 128   # 8 token chunks
    scale = float(D) ** -0.5
    order = [0, 3, 1, 4, 2, 5]   # q0,k0,q1,k1,q2,k2 chunk emission order

    with tile.TileContext(nc) as tc:
      with nc.allow_low_precision(reason="bf16 matmul intermediates"):
        with tc.tile_pool(name="consts", bufs=1) as consts, \
             tc.tile_pool(name="ins", bufs=1) as ins, \
             tc.tile_pool(name="persist", bufs=1) as persist, \
             tc.tile_pool(name="work", bufs=2) as work, \
             tc.tile_pool(name="es", bufs=16) as esp, \
             tc.tile_pool(name="avn", bufs=2) as avn, \
             tc.tile_pool(name="po", bufs=2) as pop, \
             tc.tile_pool(name="ps", bufs=2, space="PSUM") as ps:

            # ---- constants + input DMA ----
            bd = consts.tile([128, 128], BF16)
            pbt = consts.tile([128, 3], F32)
            warm_w = consts.tile([128, 64], BF16)
            ddof_b = consts.tile([128, 1], F32)
            nc.vector.memset(warm_w[:], 0.5)
            nc.vector.memset(ddof_b[:], -0.5 * float(np.log(64.0 / 63.0)))

            xt = ins.tile([128, KC, N], BF16)
            wq = ins.tile([128, KC, 3 * C], BF16)
            wp = ins.tile([128, KC, C], BF16)

            # HAM warmup gated only on the tiny ddof memset — no DMA
            # dependency, so the PE clock gate starts opening immediately.
            warm_ps = ps.tile([64, 512], F32, tag="s")
            warm_rhs = bass.AP(
                tensor=warm_w.tensor, offset=warm_w.offset,
                ap=[list(warm_w.ap[0]), [0, 8], [1, 64]])  # [128,8,64] step-0
            for _ in range(12):
                nc.tensor.matmul(warm_ps[:], warm_w[:], warm_rhs,
                                 start=True, stop=True)

            xr = xT.rearrange("(k p) n -> p k n", p=128)
            wr = qkv_wT.rearrange("(k p) m -> p k m", p=128)
            nc.scalar.dma_start(out=xt[:], in_=xr[:])
            for k in range(KC):
                nc.sync.dma_start(out=wq[:, k, 0:128], in_=wr[:, k, 0:128])
                nc.sync.dma_start(out=wq[:, k, 384:512], in_=wr[:, k, 384:512])
            nc.sync.dma_start(out=bd[:], in_=bd_in[:, :])
            nc.sync.dma_start(out=pbt[:], in_=pb[:, :])
            for k in range(KC):
                nc.sync.dma_start(out=wq[:, k, 128:384], in_=wr[:, k, 128:384])
                nc.sync.dma_start(out=wq[:, k, 512:1152], in_=wr[:, k, 512:1152])
            nc.sync.dma_start(
                out=wp[:], in_=proj_wT.rearrange("(k p) m -> p k m", p=128))

            vo = persist.tile([128, TC, H, 128], BF16)
            nc.vector.memset(vo[:, :, :, D:128], 1.0)

            qn = persist.tile([128, 2 * H, N], BF16)   # normalized q|k
            aoT = persist.tile([128, KC, N], BF16)     # attn out (proj moving)

            # ---- phase B emitters: QKV chunks + pipelined normalization ----
            qk_ps = {}      # chunk j -> psum tile with raw q|k rows
            qkr = {}        # chunk j -> bf16 copy in SBUF
            mean_ps = {}
            qc = {}
            qc2 = {}
            ssq_ps = {}

            def emit_qkv(j):
                p = ps.tile([128, N], F32, tag="s", name=f"qk_ps{j}")
                qk_ps[j] = p
                for k in range(KC):
                    for h5 in range(2):
                        nc.tensor.matmul(
                            p[:, h5 * 512:(h5 + 1) * 512],
                            wq[:, k, j * 128:(j + 1) * 128],
                            xt[:, k, h5 * 512:(h5 + 1) * 512],
                            start=(k == 0), stop=(k == KC - 1))
                # PSUM->SBUF bf16 copy on DVE (ACT is the busiest engine)
                q = work.tile([128, N], BF16, tag="qkr", name=f"qkr{j}")
                nc.vector.tensor_copy(q[:], p[:])
                qkr[j] = q

            def emit_v(t):
                v_ps = ps.tile([128, C], F32, tag="s", name=f"v_ps{t}")
                for k in range(KC):
                    nc.tensor.matmul(
                        v_ps[:],
                        xt[:, k, t * 128:(t + 1) * 128],
                        wq[:, k, 2 * C:3 * C],
                        start=(k == 0), stop=(k == KC - 1))
                nc.vector.tensor_copy(
                    vo[:, t, :, 0:D],
                    v_ps[:].rearrange("p (h d) -> p h d", h=H))

            def emit_mean(j):
                """mean broadcast + centered q, squared (DVE chain)."""
                m = ps.tile([128, N], F32, tag="av", name=f"mean_ps{j}")
                mean_ps[j] = m
                for h5 in range(2):
                    nc.tensor.matmul(m[:, h5 * 512:(h5 + 1) * 512],
                                     bd[:], qkr[j][:, h5 * 512:(h5 + 1) * 512],
                                     start=True, stop=True)
                c = work.tile([128, N], BF16, tag="qc", name=f"qc{j}")
                nc.vector.tensor_sub(c[:], qkr[j][:], m[:])
                qc[j] = c
                c2 = work.tile([128, N], BF16, tag="qc2", name=f"qc2{j}")
                nc.vector.tensor_mul(c2[:], c[:], c[:])
                qc2[j] = c2

            def emit_ssq(j):
                """ssq broadcast -> rstd (ACT ln/exp; Rsqrt is blocked in
                this bass build) -> qn (DVE mul)."""
                s = ps.tile([128, N], F32, tag="av", name=f"ssq_ps{j}")
                ssq_ps[j] = s
                for h5 in range(2):
                    nc.tensor.matmul(s[:, h5 * 512:(h5 + 1) * 512],
                                     bd[:], qc2[j][:, h5 * 512:(h5 + 1) * 512],
                                     start=True, stop=True)
                lnv = work.tile([128, N], F32, tag="lnv", name=f"lnv{j}")
                nc.scalar.activation(lnv[:], s[:], LN)
                rstd = work.tile([128, N], BF16, tag="rstd", name=f"rstd{j}")
                nc.scalar.activation(rstd[:], lnv[:], EXP, scale=-0.5,
                                     bias=ddof_b[:])
                nc.vector.tensor_mul(qn[:, j, :], qc[j][:], rstd[:])

            # pipeline: QKV(jj) | V(jj) | mean(jj-1) | ssq(jj-2)
            for jj in range(6):
                emit_qkv(order[jj])
                emit_v(jj)
                if jj >= 1:
                    emit_mean(order[jj - 1])
                if jj >= 2:
                    emit_ssq(order[jj - 2])

            # ---- pairs: QK^T -> exp -> AV, cross-pair interleaved ----
            es_tiles = {}   # (pair, t, p) -> bf16 exp(scores) tile

            def emit_qk_t(j, t):
                """Scores S^T for heads 2j,2j+1, token chunk t (64-row PE
                mode, groups T0/T8 alternating) + the exp on ACT."""
                s_tiles = {}
                for p in range(2):
                    s_tiles[p] = ps.tile([128, N], F32, tag="s",
                                         name=f"s{j}_{t}_{p}")
                for h5 in range(2):
                    for p in range(2):
                        lo = p * 64
                        nc.tensor.matmul(
                            s_tiles[p][:, h5 * 512:(h5 + 1) * 512],
                            qn[lo:lo + 64, 3 + j, t * 128:(t + 1) * 128],
                            qn[lo:lo + 64, j, h5 * 512:(h5 + 1) * 512],
                            start=True, stop=True)
                for p in range(2):
                    es = esp.tile([128, N], BF16, tag="es",
                                  name=f"es{j}_{t}_{p}")
                    nc.scalar.activation(es[:], s_tiles[p][:], EXP, scale=scale)
                    es_tiles[(j, t, p)] = es

            av_ps = {}

            def emit_av_t(j, t):
                """AV accumulation step t for pair j (128-row mode)."""
                for p in range(2):
                    if t == 0:
                        av_ps[(j, p)] = ps.tile([128, N], F32, tag="av",
                                                name=f"av{j}_{p}")
                    for h5 in range(2):
                        nc.tensor.matmul(
                            av_ps[(j, p)][:, h5 * 512:(h5 + 1) * 512],
                            vo[:, t, 2 * j + p, :],
                            es_tiles[(j, t, p)][:, h5 * 512:(h5 + 1) * 512],
                            start=(t == 0), stop=(t == TC - 1))

            def emit_norm_out(j):
                """1/denominator = exp(-ln d) on ACT, attn-out scale on DVE.
                Rows 64-127 of av_ps hold the softmax denominator already
                broadcast across 64 partitions (ones columns of vo)."""
                for p in range(2):
                    a = av_ps[(j, p)]
                    lnd = avn.tile([64, N], F32, tag="lnd", name=f"lnd{j}{p}")
                    nc.scalar.activation(lnd[:], a[D:128, :], LN)
                    rec = avn.tile([64, N], F32, tag="rec", name=f"rec{j}{p}")
                    nc.scalar.activation(rec[:], lnd[:], EXP, scale=-1.0)
                    lo = p * 64
                    nc.vector.tensor_mul(aoT[lo:lo + 64, j, :],
                                         a[0:D, :], rec[:])

            # phase B tail: all remaining Rsqrt work BEFORE the first exp
            # (one table-set switch total), then pair-0 QK^T fills the PE
            # while the last norm chains drain and the exp tables load.
            emit_v(6)
            emit_mean(order[5])
            emit_qk_t(0, 0)
            emit_v(7)
            emit_ssq(order[4])
            emit_qk_t(0, 1)
            emit_ssq(order[5])
            emit_qk_t(0, 2)
            emit_qk_t(0, 3)

            # global QK/AV software pipeline across all pairs, AV lagging 4
            # token-chunks behind QK: the PE always has a ready matmul while
            # ACT streams the exps, and pair boundaries have no PE gap.
            steps = [(j, t) for j in range(3) for t in range(TC)]
            LAG = 5
            proj_ps = {}
            for s in range(4, len(steps) + LAG):
                if s < len(steps):
                    emit_qk_t(*steps[s])
                if s < LAG:
                    continue
                if s == len(steps) + LAG - 2:
                    # pre-accumulate proj contractions k=0,1 for the first
                    # two output chunks (pairs 0,1 attn-out already final);
                    # co=2 stays fully in the tail so the 2-slot "s" tag
                    # rotation can't WAR-deadlock the in-order PE queue.
                    for co in range(2):
                        p_ps = ps.tile([128, N], F32, tag="s",
                                       name=f"p_ps{co}")
                        proj_ps[co] = p_ps
                        for h5 in range(2):
                            for k in range(2):
                                nc.tensor.matmul(
                                    p_ps[:, h5 * 512:(h5 + 1) * 512],
                                    wp[:, k, co * 128:(co + 1) * 128],
                                    aoT[:, k, h5 * 512:(h5 + 1) * 512],
                                    start=(k == 0), stop=False)
                j, t = steps[s - LAG]
                emit_av_t(j, t)
                if t == TC - 1:
                    emit_norm_out(j)

            # ---- output projection tail ----
            for co in range(2):
                p_ps = proj_ps[co]
                for h5 in range(2):
                    nc.tensor.matmul(
                        p_ps[:, h5 * 512:(h5 + 1) * 512],
                        wp[:, 2, co * 128:(co + 1) * 128],
                        aoT[:, 2, h5 * 512:(h5 + 1) * 512],
                        start=False, stop=True)
                po = pop.tile([128, N], BF16, tag="po", name=f"po{co}")
                nc.vector.tensor_scalar_add(po[:], p_ps[:], pbt[:, co:co + 1])
                nc.sync.dma_start(out=outT[co * 128:(co + 1) * 128, :], in_=po[:])
            p_ps2 = ps.tile([128, N], F32, tag="s", name="p_ps2")
            for h5 in range(2):
                for k in range(KC):
                    nc.tensor.matmul(
                        p_ps2[:, h5 * 512:(h5 + 1) * 512],
                        wp[:, k, 2 * 128:3 * 128],
                        aoT[:, k, h5 * 512:(h5 + 1) * 512],
                        start=(k == 0), stop=(k == KC - 1))
            po2 = pop.tile([128, N], BF16, tag="po", name="po2")
            nc.vector.tensor_scalar_add(po2[:], p_ps2[:], pbt[:, 2:3])
            nc.sync.dma_start(out=outT[2 * 128:3 * 128, :], in_=po2[:])

    return nc


def _get_prog():
    global _prog
    if _prog is None:
        _prog = _build()
    return _prog


def _make_in_maps(x, qkv_w, proj_w, proj_b):
    from ml_dtypes import bfloat16

    qkv_wT = np.ascontiguousarray(np.asarray(qkv_w, np.float32).T).astype(bfloat16)
    proj_wT = np.ascontiguousarray(np.asarray(proj_w, np.float32).T).astype(bfloat16)
    pb = np.ascontiguousarray(
        np.asarray(proj_b, np.float32).reshape(3, 128).T)
    bd_in = np.zeros((128, 128), np.float32)
    for b0 in (0, 64):
        bd_in[b0:b0 + 64, b0:b0 + 64] = 1.0 / D   # ddof fix in rstd exp bias
    bd_in = bd_in.astype(bfloat16)

    shared = {
        "qkv_wT": qkv_wT, "proj_wT": proj_wT, "pb": pb, "bd_in": bd_in,
    }
    x = np.asarray(x, np.float32)
    return [
        {"xT": np.ascontiguousarray(x[b].T).astype(bfloat16), **shared}
        for b in range(B)
    ]


def run(x, qkv_w, proj_w, proj_b, trace=False):
    from concourse.bass_utils import run_bass_kernel_spmd

    nc = _get_prog()
    in_maps = _make_in_maps(x, qkv_w, proj_w, proj_b)
    res = run_bass_kernel_spmd(
        nc, in_maps, core_ids=list(range(NCORES)), trace=trace)
    out = np.stack(
        [res.results[b]["outT"].astype(np.float32).T for b in range(B)])
    return np.ascontiguousarray(out.astype(np.float32)), res


def kernel(x, qkv_w, proj_w, proj_b):
    out, _ = run(x, qkv_w, proj_w, proj_b)
    return out
